# revision 6
# baseline (speedup 1.0000x reference)
"""Trainium2 Bass kernel for nn_Attention_41085657153633.

B=8, N=1024, C=384, H=6, D=64 attention with per-head q/k normalization
(mean/std over head_dim, ddof=1), softmax(QK^T/sqrt(D)) @ V, output proj.

Sharding: data-parallel over B — each of the 8 NeuronCores handles one
batch element end-to-end (no collectives).

Per-core dataflow (all matmuls fp32r unless noted):
  - host supplies x[b]^T [384,1024]; QKV^T computed head-major for Q,K
    ([d on partitions, tokens on free] — what QK^T wants as lhsT/rhs)
    and token-major for V (what AV wants as lhsT), V interleaved with a
    ones column per head so the AV matmul's row 0 accumulates the
    softmax denominator for free.
  - q/k normalization over the partition (d) axis via block-diagonal
    ones matmuls: mean_bcast = blockdiag(1/64) @ q, ssq_bcast =
    blockdiag(1/63) @ (q-mean)^2, both broadcast across the 64
    partitions of each head by construction; rstd via ACT Rsqrt.
  - scores S^T[s,t] per head via K^T x Q^T (contraction over d=64);
    heads are processed in pairs occupying PE row groups 0-63/64-127 so
    the two K=64 matmuls run concurrently in the array.
  - exp on ACT with the 1/sqrt(D) scale folded in (softmax max-sub
    skipped: |S/8| <~ 7, safe in fp32); bf16 scores/V for the AV stage.
  - denominators broadcast across partitions with a rank-1 ones matmul;
    one DVE multiply normalizes the AV output.
  - output proj from attn_out^T; host transposes the [384,1024] result.
"""

import sys

sys.path.insert(0, "/opt/trn_rl_repo")

import json

import numpy as np

B, N, C = 8, 1024, 384
H, D = 6, 64
NCORES = 8

_prog = None


def _install_multiwait_fixup():
    """This container's walrus build rejects >1 sync wait per instruction
    ("Too many sync wait commands"). Rewrite the BIR JSON before compile:
    hoist extra waits onto single-wait EventSemaphore instructions
    inserted just before the owner on the same engine (engines dispatch
    in program order, so the gating is preserved)."""
    from concourse import bass2jax, bass_utils

    if getattr(bass_utils, "_multiwait_fixup", False):
        return
    bass_utils._multiwait_fixup = True

    orig = bass_utils.compile_bir_kernel

    def _split(bir_json: bytes) -> bytes:
        j = json.loads(bir_json)
        for fn in j.get("functions", []):
            for bb in fn.get("blocks", []):
                out = []
                for inst in bb.get("instructions", []):
                    si = inst.get("sync_info")
                    waits = si.get("on_wait", []) if si else []
                    if len(waits) > 1:
                        for k, w in enumerate(waits[:-1]):
                            out.append({
                                "debug": inst.get("debug", 0),
                                "engine": inst["engine"],
                                "ins": [],
                                "outs": [],
                                "name": f"{inst['name']}-sw{k}",
                                "opcode": "EventSemaphore",
                                "sync_info": {"on_update": [], "on_wait": [w]},
                            })
                        si["on_wait"] = [waits[-1]]
                    out.append(inst)
                bb["instructions"] = out
        return json.dumps(j).encode()

    def patched(bir_json, tmpdir, neff_name="file.neff"):
        return orig(_split(bir_json), tmpdir, neff_name)

    bass_utils.compile_bir_kernel = patched
    bass2jax.compile_bir_kernel = patched


def _build():
    import concourse.bass as bass
    import concourse.tile as tile
    from concourse import mybir

    _install_multiwait_fixup()

    F32 = mybir.dt.float32
    F32R = mybir.dt.float32r
    BF16 = mybir.dt.bfloat16
    EXP = mybir.ActivationFunctionType.Exp
    LN = mybir.ActivationFunctionType.Ln

    nc = bass.Bass("TRN2")
    xT = nc.dram_tensor("xT", [C, N], F32, kind="ExternalInput")
    qkv_wT = nc.dram_tensor("qkv_wT", [C, 3 * C], F32, kind="ExternalInput")
    proj_wT = nc.dram_tensor("proj_wT", [C, C], F32, kind="ExternalInput")
    pb = nc.dram_tensor("pb", [128, 3], F32, kind="ExternalInput")
    bd_mean = nc.dram_tensor("bd_mean", [128, 128], F32, kind="ExternalInput")
    bd_ssq = nc.dram_tensor("bd_ssq", [128, 128], F32, kind="ExternalInput")
    ones_tc = nc.dram_tensor("ones_tc", [128, 48], BF16, kind="ExternalInput")
    ones64 = nc.dram_tensor("ones64", [1, 64], F32, kind="ExternalInput")
    outT = nc.dram_tensor("outT", [C, N], F32, kind="ExternalOutput")

    KC = C // 128   # 3 contraction chunks of the model dim
    TC = N // 128   # 8 token chunks
    HALF = N // 512  # 2 free-dim halves per 1024-token row

    with tile.TileContext(nc) as tc:
      with nc.allow_low_precision(reason="f32r/bf16 matmul intermediates"):
        with tc.tile_pool(name="consts", bufs=1) as consts, \
             tc.tile_pool(name="ins", bufs=1) as ins, \
             tc.tile_pool(name="persist", bufs=1) as persist, \
             tc.tile_pool(name="work", bufs=2) as work, \
             tc.tile_pool(name="es", bufs=10) as esp, \
             tc.tile_pool(name="avn", bufs=2) as avn, \
             tc.tile_pool(name="po", bufs=2) as pop, \
             tc.tile_pool(name="ps", bufs=2, space="PSUM") as ps, \
             tc.tile_pool(name="avps", bufs=2, space="PSUM") as avps:

            # ---- loads ----
            bdm = consts.tile([128, 128], F32R)
            bds = consts.tile([128, 128], F32R)
            pbt = consts.tile([128, 3], F32)
            o64 = consts.tile([1, 64], F32R)
            nc.sync.dma_start(out=bdm[:], in_=bd_mean[:, :].bitcast(F32R))
            nc.sync.dma_start(out=bds[:], in_=bd_ssq[:, :].bitcast(F32R))
            nc.sync.dma_start(out=pbt[:], in_=pb[:, :])
            nc.sync.dma_start(out=o64[:], in_=ones64[:, :].bitcast(F32R))

            xt = ins.tile([128, KC, N], F32R)
            wq = ins.tile([128, KC, 3 * C], F32R)
            wp = ins.tile([128, KC, C], F32R)
            nc.sync.dma_start(
                out=xt[:], in_=xT.rearrange("(k p) n -> p k n", p=128).bitcast(F32R))
            nc.sync.dma_start(
                out=wq[:], in_=qkv_wT.rearrange("(k p) m -> p k m", p=128).bitcast(F32R))
            nc.sync.dma_start(
                out=wp[:], in_=proj_wT.rearrange("(k p) m -> p k m", p=128).bitcast(F32R))

            vo = persist.tile([128, TC, H, D + 1], BF16)
            nc.sync.dma_start(
                out=vo[:, :, :, D:D + 1],
                in_=ones_tc[:, :].rearrange("p (a b) -> p a b", a=TC).unsqueeze(-1))

            # ---- QKV (Q,K head-major) fused with q/k normalization ----
            qn = persist.tile([128, 2 * H, N], F32R)    # normalized q|k
            for j in range(6):  # 0-2 = Q chunks, 3-5 = K chunks
                qk_ps = ps.tile([128, N], F32, tag="big")
                for h5 in range(HALF):
                    for k in range(KC):
                        nc.tensor.matmul(
                            qk_ps[:, h5 * 512:(h5 + 1) * 512],
                            wq[:, k, j * 128:(j + 1) * 128],
                            xt[:, k, h5 * 512:(h5 + 1) * 512],
                            start=(k == 0), stop=(k == KC - 1))
                qkr = work.tile([128, N], F32R, tag="qkr")
                nc.vector.tensor_copy(qkr[:], qk_ps[:])
                mean_ps = ps.tile([128, N], F32, tag="big")
                for h5 in range(HALF):
                    nc.tensor.matmul(
                        mean_ps[:, h5 * 512:(h5 + 1) * 512], bdm[:],
                        qkr[:, h5 * 512:(h5 + 1) * 512], start=True, stop=True)
                qc = work.tile([128, N], F32R, tag="qc")
                nc.vector.tensor_sub(qc[:], qkr[:], mean_ps[:])
                qc2 = work.tile([128, N], F32R, tag="qc2")
                nc.vector.tensor_mul(qc2[:], qc[:], qc[:])
                ssq_ps = ps.tile([128, N], F32, tag="big")
                for h5 in range(HALF):
                    nc.tensor.matmul(
                        ssq_ps[:, h5 * 512:(h5 + 1) * 512], bds[:],
                        qc2[:, h5 * 512:(h5 + 1) * 512], start=True, stop=True)
                # rstd = exp(-0.5*ln(var)) — Rsqrt/Reciprocal ACT tables are
                # blocked for accuracy, and ln/exp share the softmax's table set
                lnv = work.tile([128, N], F32R, tag="qc2")
                nc.scalar.activation(lnv[:], ssq_ps[:], LN)
                rstd = work.tile([128, N], F32R, tag="rstd")
                nc.scalar.activation(rstd[:], lnv[:], EXP, scale=-0.5)
                nc.vector.tensor_mul(qn[:, j, :], qc[:], rstd[:])

            # ---- QKV: V token-major, interleaved with ones columns ----
            for t in range(TC):
                v_ps = ps.tile([128, C], F32, tag="big")
                for k in range(KC):
                    nc.tensor.matmul(
                        v_ps[:],
                        xt[:, k, t * 128:(t + 1) * 128],
                        wq[:, k, 2 * C:3 * C],
                        start=(k == 0), stop=(k == KC - 1))
                nc.vector.tensor_copy(
                    vo[:, t, :, 0:D],
                    v_ps[:].rearrange("p (h d) -> p h d", h=H))

            # ---- attention, head pairs on PE row groups 0-63 / 64-127 ----
            aoT = persist.tile([128, KC, N], F32R)
            scale = float(D) ** -0.5
            for j in range(3):
                es_tiles = {0: [], 1: []}
                s_tiles = {}
                for t in range(TC):
                    for p in range(2):  # even/odd head of the pair
                        lo = p * 64
                        s_ps = ps.tile([128, N], F32, tag="big")
                        for h5 in range(HALF):
                            nc.tensor.matmul(
                                s_ps[:, h5 * 512:(h5 + 1) * 512],
                                qn[lo:lo + 64, 3 + j, t * 128:(t + 1) * 128],
                                qn[lo:lo + 64, j, h5 * 512:(h5 + 1) * 512],
                                start=True, stop=True)
                        s_tiles[p] = s_ps
                    for p in range(2):
                        es = esp.tile([128, N], BF16, tag="es")
                        nc.scalar.activation(es[:], s_tiles[p][:], EXP, scale=scale)
                        es_tiles[p].append(es)
                for p in range(2):
                    h = 2 * j + p
                    av_ps = avps.tile([D + 1, N], F32, tag="av")
                    for t in range(TC):
                        for h5 in range(HALF):
                            nc.tensor.matmul(
                                av_ps[:, h5 * 512:(h5 + 1) * 512],
                                vo[:, t, h, :],
                                es_tiles[p][t][:, h5 * 512:(h5 + 1) * 512],
                                start=(t == 0), stop=(t == TC - 1))
                    recip = avn.tile([1, N], F32R, tag="recip")
                    nc.vector.reciprocal(recip[:], av_ps[D:D + 1, :])
                    bc_ps = ps.tile([64, N], F32, tag="big")
                    for h5 in range(HALF):
                        nc.tensor.matmul(
                            bc_ps[:, h5 * 512:(h5 + 1) * 512], o64[:],
                            recip[:, h5 * 512:(h5 + 1) * 512], start=True, stop=True)
                    av_sb = avn.tile([64, N], F32R, tag="av_sb")
                    nc.vector.tensor_copy(av_sb[:], av_ps[0:D, :])
                    lo = p * 64
                    nc.vector.tensor_mul(aoT[lo:lo + 64, j, :], av_sb[:], bc_ps[:])

            # ---- output projection ----
            for co in range(KC):
                p_ps = ps.tile([128, N], F32, tag="big")
                for h5 in range(HALF):
                    for k in range(KC):
                        nc.tensor.matmul(
                            p_ps[:, h5 * 512:(h5 + 1) * 512],
                            wp[:, k, co * 128:(co + 1) * 128],
                            aoT[:, k, h5 * 512:(h5 + 1) * 512],
                            start=(k == 0), stop=(k == KC - 1))
                po = pop.tile([128, N], F32, tag="po")
                nc.vector.tensor_scalar_add(po[:], p_ps[:], pbt[:, co:co + 1])
                nc.sync.dma_start(out=outT[co * 128:(co + 1) * 128, :], in_=po[:])

    return nc


def _get_prog():
    global _prog
    if _prog is None:
        _prog = _build()
    return _prog


def _make_in_maps(x, qkv_w, proj_w, proj_b):
    import ml_dtypes

    qkv_wT = np.ascontiguousarray(np.asarray(qkv_w, np.float32).T)
    proj_wT = np.ascontiguousarray(np.asarray(proj_w, np.float32).T)
    pb = np.ascontiguousarray(
        np.asarray(proj_b, np.float32).reshape(3, 128).T)
    bd_mean = np.zeros((128, 128), np.float32)
    bd_ssq = np.zeros((128, 128), np.float32)
    for b0 in (0, 64):
        bd_mean[b0:b0 + 64, b0:b0 + 64] = 1.0 / D
        bd_ssq[b0:b0 + 64, b0:b0 + 64] = 1.0 / (D - 1)
    ones_tc = np.ones((128, 48), ml_dtypes.bfloat16)
    ones64 = np.ones((1, 64), np.float32)
    shared = {
        "qkv_wT": qkv_wT, "proj_wT": proj_wT, "pb": pb,
        "bd_mean": bd_mean, "bd_ssq": bd_ssq,
        "ones_tc": ones_tc, "ones64": ones64,
    }
    x = np.asarray(x, np.float32)
    return [
        {"xT": np.ascontiguousarray(x[b].T), **shared} for b in range(B)
    ]


def run(x, qkv_w, proj_w, proj_b, trace=False):
    from concourse.bass_utils import run_bass_kernel_spmd

    nc = _get_prog()
    in_maps = _make_in_maps(x, qkv_w, proj_w, proj_b)
    res = run_bass_kernel_spmd(
        nc, in_maps, core_ids=list(range(NCORES)), trace=trace)
    out = np.stack([res.results[b]["outT"].T for b in range(B)])
    return np.ascontiguousarray(out.astype(np.float32)), res


def kernel(x, qkv_w, proj_w, proj_b):
    out, _ = run(x, qkv_w, proj_w, proj_b)
    return out


# revision 7
# speedup vs baseline: 1.2429x; 1.2429x over previous
"""Trainium2 Bass kernel for nn_Attention_41085657153633.

B=8, N=1024, C=384, H=6, D=64 attention with per-head q/k normalization
(mean/std over head_dim, ddof=1), softmax(QK^T/sqrt(D)) @ V, output proj.

Sharding: data-parallel over B — each of the 8 NeuronCores handles one
batch element end-to-end (no collectives).

Per-core dataflow (all matmuls fp32r unless noted):
  - host supplies x[b]^T [384,1024]; QKV^T computed head-major for Q,K
    ([d on partitions, tokens on free] — what QK^T wants as lhsT/rhs)
    and token-major for V (what AV wants as lhsT), V interleaved with a
    ones column per head so the AV matmul's row 0 accumulates the
    softmax denominator for free.
  - q/k normalization over the partition (d) axis via block-diagonal
    ones matmuls: mean_bcast = blockdiag(1/64) @ q, ssq_bcast =
    blockdiag(1/63) @ (q-mean)^2, both broadcast across the 64
    partitions of each head by construction; rstd via ACT Rsqrt.
  - scores S^T[s,t] per head via K^T x Q^T (contraction over d=64);
    heads are processed in pairs occupying PE row groups 0-63/64-127 so
    the two K=64 matmuls run concurrently in the array.
  - exp on ACT with the 1/sqrt(D) scale folded in (softmax max-sub
    skipped: |S/8| <~ 7, safe in fp32); bf16 scores/V for the AV stage.
  - denominators broadcast across partitions with a rank-1 ones matmul;
    one DVE multiply normalizes the AV output.
  - output proj from attn_out^T; host transposes the [384,1024] result.
"""

import sys

sys.path.insert(0, "/opt/trn_rl_repo")

import json

import numpy as np

B, N, C = 8, 1024, 384
H, D = 6, 64
NCORES = 8

_prog = None


def _install_multiwait_fixup():
    """This container's walrus build rejects >1 sync wait per instruction
    ("Too many sync wait commands"). Rewrite the BIR JSON before compile:
    hoist extra waits onto single-wait EventSemaphore instructions
    inserted just before the owner on the same engine (engines dispatch
    in program order, so the gating is preserved)."""
    from concourse import bass2jax, bass_utils

    if getattr(bass_utils, "_multiwait_fixup", False):
        return
    bass_utils._multiwait_fixup = True

    orig = bass_utils.compile_bir_kernel

    def _split(bir_json: bytes) -> bytes:
        j = json.loads(bir_json)
        for fn in j.get("functions", []):
            for bb in fn.get("blocks", []):
                out = []
                for inst in bb.get("instructions", []):
                    si = inst.get("sync_info")
                    waits = si.get("on_wait", []) if si else []
                    if len(waits) > 1:
                        for k, w in enumerate(waits[:-1]):
                            out.append({
                                "debug": inst.get("debug", 0),
                                "engine": inst["engine"],
                                "ins": [],
                                "outs": [],
                                "name": f"{inst['name']}-sw{k}",
                                "opcode": "EventSemaphore",
                                "sync_info": {"on_update": [], "on_wait": [w]},
                            })
                        si["on_wait"] = [waits[-1]]
                    out.append(inst)
                bb["instructions"] = out
        return json.dumps(j).encode()

    def patched(bir_json, tmpdir, neff_name="file.neff"):
        return orig(_split(bir_json), tmpdir, neff_name)

    bass_utils.compile_bir_kernel = patched
    bass2jax.compile_bir_kernel = patched


def _build():
    import concourse.bass as bass
    import concourse.tile as tile
    from concourse import mybir

    _install_multiwait_fixup()

    F32 = mybir.dt.float32
    F32R = mybir.dt.float32r
    BF16 = mybir.dt.bfloat16
    EXP = mybir.ActivationFunctionType.Exp
    LN = mybir.ActivationFunctionType.Ln

    nc = bass.Bass("TRN2")
    xT = nc.dram_tensor("xT", [C, N], F32, kind="ExternalInput")
    qkv_wT = nc.dram_tensor("qkv_wT", [C, 3 * C], F32, kind="ExternalInput")
    proj_wT = nc.dram_tensor("proj_wT", [C, C], F32, kind="ExternalInput")
    pb = nc.dram_tensor("pb", [128, 3], F32, kind="ExternalInput")
    bd_mean = nc.dram_tensor("bd_mean", [128, 128], F32, kind="ExternalInput")
    bd_ssq = nc.dram_tensor("bd_ssq", [128, 128], F32, kind="ExternalInput")
    outT = nc.dram_tensor("outT", [C, N], F32, kind="ExternalOutput")

    KC = C // 128   # 3 contraction chunks of the model dim
    TC = N // 128   # 8 token chunks
    HALF = N // 512  # 2 free-dim halves per 1024-token row

    with tile.TileContext(nc) as tc:
      with nc.allow_low_precision(reason="f32r/bf16 matmul intermediates"):
        with tc.tile_pool(name="consts", bufs=1) as consts, \
             tc.tile_pool(name="ins", bufs=1) as ins, \
             tc.tile_pool(name="persist", bufs=1) as persist, \
             tc.tile_pool(name="work", bufs=2) as work, \
             tc.tile_pool(name="es", bufs=10) as esp, \
             tc.tile_pool(name="avn", bufs=2) as avn, \
             tc.tile_pool(name="po", bufs=2) as pop, \
             tc.tile_pool(name="ps", bufs=2, space="PSUM") as ps, \
             tc.tile_pool(name="avps", bufs=2, space="PSUM") as avps:

            # ---- loads ----
            bdm = consts.tile([128, 128], F32R)
            bds = consts.tile([128, 128], F32R)
            pbt = consts.tile([128, 3], F32)
            nc.sync.dma_start(out=bdm[:], in_=bd_mean[:, :].bitcast(F32R))
            nc.sync.dma_start(out=bds[:], in_=bd_ssq[:, :].bitcast(F32R))
            nc.sync.dma_start(out=pbt[:], in_=pb[:, :])

            xt = ins.tile([128, KC, N], F32R)
            wq = ins.tile([128, KC, 3 * C], F32R)
            wp = ins.tile([128, KC, C], F32R)
            nc.sync.dma_start(
                out=xt[:], in_=xT.rearrange("(k p) n -> p k n", p=128).bitcast(F32R))
            nc.sync.dma_start(
                out=wq[:], in_=qkv_wT.rearrange("(k p) m -> p k m", p=128).bitcast(F32R))
            nc.sync.dma_start(
                out=wp[:], in_=proj_wT.rearrange("(k p) m -> p k m", p=128).bitcast(F32R))

            vo = persist.tile([128, TC, H, 128], BF16)
            nc.vector.memset(vo[:, :, :, D:128], 1.0)

            # ---- QKV (Q,K head-major) fused with q/k normalization ----
            qn = persist.tile([128, 2 * H, N], BF16)    # normalized q|k (bf16 -> FWL)
            for j in range(6):  # 0-2 = Q chunks, 3-5 = K chunks
                qk_ps = ps.tile([128, N], F32, tag="big")
                for k in range(KC):
                    for h5 in range(HALF):
                        nc.tensor.matmul(
                            qk_ps[:, h5 * 512:(h5 + 1) * 512],
                            wq[:, k, j * 128:(j + 1) * 128],
                            xt[:, k, h5 * 512:(h5 + 1) * 512],
                            start=(k == 0), stop=(k == KC - 1))
                qkr = work.tile([128, N], F32R, tag="qkr")
                nc.vector.tensor_copy(qkr[:], qk_ps[:])
                mean_ps = ps.tile([128, N], F32, tag="big")
                for h5 in range(HALF):
                    nc.tensor.matmul(
                        mean_ps[:, h5 * 512:(h5 + 1) * 512], bdm[:],
                        qkr[:, h5 * 512:(h5 + 1) * 512], start=True, stop=True)
                qc = work.tile([128, N], F32R, tag="qc")
                nc.vector.tensor_sub(qc[:], qkr[:], mean_ps[:])
                qc2 = work.tile([128, N], F32R, tag="qc2")
                nc.vector.tensor_mul(qc2[:], qc[:], qc[:])
                ssq_ps = ps.tile([128, N], F32, tag="big")
                for h5 in range(HALF):
                    nc.tensor.matmul(
                        ssq_ps[:, h5 * 512:(h5 + 1) * 512], bds[:],
                        qc2[:, h5 * 512:(h5 + 1) * 512], start=True, stop=True)
                # rstd = exp(-0.5*ln(var)) — Rsqrt/Reciprocal ACT tables are
                # blocked for accuracy, and ln/exp share the softmax's table set
                lnv = work.tile([128, N], F32R, tag="qc2")
                nc.scalar.activation(lnv[:], ssq_ps[:], LN)
                rstd = work.tile([128, N], F32R, tag="rstd")
                nc.scalar.activation(rstd[:], lnv[:], EXP, scale=-0.5)
                nc.vector.tensor_mul(qn[:, j, :], qc[:], rstd[:])

            # ---- QKV: V token-major, interleaved with ones columns ----
            for t in range(TC):
                v_ps = ps.tile([128, C], F32, tag="big")
                for k in range(KC):
                    nc.tensor.matmul(
                        v_ps[:],
                        xt[:, k, t * 128:(t + 1) * 128],
                        wq[:, k, 2 * C:3 * C],
                        start=(k == 0), stop=(k == KC - 1))
                nc.vector.tensor_copy(
                    vo[:, t, :, 0:D],
                    v_ps[:].rearrange("p (h d) -> p h d", h=H))

            # ---- attention, head pairs on PE row groups 0-63 / 64-127 ----
            aoT = persist.tile([128, KC, N], F32R)
            scale = float(D) ** -0.5
            for j in range(3):
                es_tiles = {0: [], 1: []}
                s_tiles = {}
                for t in range(TC):
                    for p in range(2):  # even/odd head of the pair
                        lo = p * 64
                        s_ps = ps.tile([128, N], F32, tag="big")
                        for h5 in range(HALF):
                            nc.tensor.matmul(
                                s_ps[:, h5 * 512:(h5 + 1) * 512],
                                qn[lo:lo + 64, 3 + j, t * 128:(t + 1) * 128],
                                qn[lo:lo + 64, j, h5 * 512:(h5 + 1) * 512],
                                start=True, stop=True)
                        s_tiles[p] = s_ps
                    for p in range(2):
                        es = esp.tile([128, N], BF16, tag="es")
                        nc.scalar.activation(es[:], s_tiles[p][:], EXP, scale=scale)
                        es_tiles[p].append(es)
                for p in range(2):
                    h = 2 * j + p
                    av_ps = avps.tile([128, N], F32, tag="av")
                    for t in range(TC):
                        for h5 in range(HALF):
                            nc.tensor.matmul(
                                av_ps[:, h5 * 512:(h5 + 1) * 512],
                                vo[:, t, h, :],
                                es_tiles[p][t][:, h5 * 512:(h5 + 1) * 512],
                                start=(t == 0), stop=(t == TC - 1))
                    # rows 64-127 hold the softmax denominator already broadcast
                    # across 64 partitions; 1/d via exp(-ln d) (ACT, one table set)
                    lnd = avn.tile([64, N], F32R, tag="lnd")
                    nc.scalar.activation(lnd[:], av_ps[D:128, :], LN)
                    rec_b = avn.tile([64, N], F32R, tag="rec_b")
                    nc.scalar.activation(rec_b[:], lnd[:], EXP, scale=-1.0)
                    lo = p * 64
                    nc.vector.tensor_mul(aoT[lo:lo + 64, j, :], av_ps[0:D, :], rec_b[:])

            # ---- output projection ----
            for co in range(KC):
                p_ps = ps.tile([128, N], F32, tag="big")
                for h5 in range(HALF):
                    for k in range(KC):
                        nc.tensor.matmul(
                            p_ps[:, h5 * 512:(h5 + 1) * 512],
                            wp[:, k, co * 128:(co + 1) * 128],
                            aoT[:, k, h5 * 512:(h5 + 1) * 512],
                            start=(k == 0), stop=(k == KC - 1))
                po = pop.tile([128, N], F32, tag="po")
                nc.vector.tensor_scalar_add(po[:], p_ps[:], pbt[:, co:co + 1])
                nc.sync.dma_start(out=outT[co * 128:(co + 1) * 128, :], in_=po[:])

    return nc


def _get_prog():
    global _prog
    if _prog is None:
        _prog = _build()
    return _prog


def _make_in_maps(x, qkv_w, proj_w, proj_b):
    import ml_dtypes

    qkv_wT = np.ascontiguousarray(np.asarray(qkv_w, np.float32).T)
    proj_wT = np.ascontiguousarray(np.asarray(proj_w, np.float32).T)
    pb = np.ascontiguousarray(
        np.asarray(proj_b, np.float32).reshape(3, 128).T)
    bd_mean = np.zeros((128, 128), np.float32)
    bd_ssq = np.zeros((128, 128), np.float32)
    for b0 in (0, 64):
        bd_mean[b0:b0 + 64, b0:b0 + 64] = 1.0 / D
        bd_ssq[b0:b0 + 64, b0:b0 + 64] = 1.0 / (D - 1)
    shared = {
        "qkv_wT": qkv_wT, "proj_wT": proj_wT, "pb": pb,
        "bd_mean": bd_mean, "bd_ssq": bd_ssq,
    }
    x = np.asarray(x, np.float32)
    return [
        {"xT": np.ascontiguousarray(x[b].T), **shared} for b in range(B)
    ]


def run(x, qkv_w, proj_w, proj_b, trace=False):
    from concourse.bass_utils import run_bass_kernel_spmd

    nc = _get_prog()
    in_maps = _make_in_maps(x, qkv_w, proj_w, proj_b)
    res = run_bass_kernel_spmd(
        nc, in_maps, core_ids=list(range(NCORES)), trace=trace)
    out = np.stack([res.results[b]["outT"].T for b in range(B)])
    return np.ascontiguousarray(out.astype(np.float32)), res


def kernel(x, qkv_w, proj_w, proj_b):
    out, _ = run(x, qkv_w, proj_w, proj_b)
    return out
